# revision 17
# baseline (speedup 1.0000x reference)
"""Trainium2 Bass kernel for a 2-layer GCN (PyG GCNConv semantics).

Strategy (8 NeuronCores, SPMD, full I/O):
  - Destinations partitioned contiguously: 100 blocks of 128 dst nodes per
    core (N_PAD = 102400). Sources split into 4 groups of 25600 rows so
    int16 indices work with dma_gather (4 SWDGE queues, one per group).
  - Every (block, group) cell holds EXACTLY 512 edges: short cells are
    padded with dummy (idx 0, w 0) entries, overflowing edges (~2.4%) are
    spilled to the host. Fixed counts mean no -1 index skips, no count
    registers, and gathers batch 4 blocks per instruction (SWDGE prep
    amortized 16x vs per-cell gathers).
  - The one-hot scatter matrices (one [512e x 128n] tile set per block,
    identical across both layers) are split: SPLIT tiles are built ON CHIP
    per block from 4 B/edge of metadata via two DVE broadcast passes
    (S0 = (iota == slot), S = S0 * w), the remaining tiles plus the
    self-loop diagonal are host-built and streamed from DRAM. The split
    balances the DVE (1 elem/cycle on broadcast-AP passes) against the
    leftover DMA bandwidth.
  - Per block: PSUM[f, n] += sum_t G_t^T @ S_t (+ self-loop tile), then
    psum2[fo, n] = W^T @ agg (lhsT = W so the bias lands on partitions) and
    out = psum2 + b via one ScalarE Identity activation. Outputs are
    PRE-relu so the host can add the spilled edges' contribution before
    applying relu between layers.
  - Two launches (one per GCN layer) of the same compiled program.
"""

import os
from contextlib import ExitStack

import numpy as np

import concourse.bacc as bacc
import concourse.bass as bass
import concourse.mybir as mybir
import concourse.tile as tile
from concourse import bass_utils

P = 128
D = 128
NCORES = 8
NB = 100                  # dst blocks per core
SHARD = NB * P            # 12800
N_PAD = SHARD * NCORES    # 102400
NGROUP = 4
GROWS = N_PAD // NGROUP   # 25600 (fits int16)
TPG = 4                   # gather tiles per (block, group) cell
CAP = TPG * P             # 512 edges per cell, exact
SB = 4                    # blocks per super-block (one gather instr each)
NSB = NB // SB            # 25
NIDX = SB * CAP           # 2048 idxs per gather instruction
GTILES = SB * TPG         # 16 tiles per G buffer
TB = NGROUP * TPG         # 16 aggregation tiles per block
SPLIT = 11                # S tiles built on DVE; TB-SPLIT+1 streamed
NSTR = TB - SPLIT + 1     # streamed tiles per block (incl. diagonal)
MCOLS = 2 * SPLIT         # slot[SPLIT] w[SPLIT] metadata cols per block
N_NODES = 100000

_nc_cache = {}


def build_nc():
    dt = mybir.dt
    nc = bacc.Bacc(
        "TRN2",
        target_bir_lowering=False,
        debug=False,
        enable_asserts=False,
        num_devices=1,
        num_swdge_queues=4,
    )
    zt = nc.dram_tensor("zt", [N_PAD, D], dt.float16, kind="ExternalInput")
    ixd = nc.dram_tensor("ixd", [NSB, P, NGROUP * P], dt.int16,
                         kind="ExternalInput")
    meta = nc.dram_tensor("meta", [NSB, P, SB * MCOLS], dt.float16,
                          kind="ExternalInput")
    sdram = nc.dram_tensor("sdram", [NSB, P, SB * NSTR * P], dt.float16,
                           kind="ExternalInput")
    zsb = nc.dram_tensor("zsb", [NSB, P, SB * P], dt.float16,
                         kind="ExternalInput")
    wt = nc.dram_tensor("wt", [D, D], dt.float16, kind="ExternalInput")
    bcol = nc.dram_tensor("bcol", [D, 1], dt.float32, kind="ExternalInput")
    iot = nc.dram_tensor("iot", [P, P], dt.float16, kind="ExternalInput")
    out = nc.dram_tensor("out", [D, NB * P], dt.float16, kind="ExternalOutput")

    eq = mybir.AluOpType.is_equal
    mul = mybir.AluOpType.mult

    with tile.TileContext(nc) as tc, ExitStack() as ctx:
        const = ctx.enter_context(tc.tile_pool(name="const", bufs=1))
        ixpool = ctx.enter_context(tc.tile_pool(name="ix", bufs=4))
        mpool = ctx.enter_context(tc.tile_pool(name="mt", bufs=4))
        zspool = ctx.enter_context(tc.tile_pool(name="zs", bufs=4))
        strpool = ctx.enter_context(tc.tile_pool(name="sst", bufs=4))
        gpools = [
            ctx.enter_context(tc.tile_pool(name=f"g{g}", bufs=5))
            for g in range(NGROUP)
        ]
        spool = ctx.enter_context(tc.tile_pool(name="s", bufs=14))
        apool = ctx.enter_context(tc.tile_pool(name="agg", bufs=6))
        opool = ctx.enter_context(tc.tile_pool(name="o", bufs=4))
        ppool = ctx.enter_context(tc.tile_pool(name="ps", bufs=4, space="PSUM"))
        p2pool = ctx.enter_context(tc.tile_pool(name="ps2", bufs=4, space="PSUM"))

        w_t = const.tile([D, D], dt.float16)
        nc.sync.dma_start(out=w_t[:], in_=wt[:])
        b_t = const.tile([D, 1], dt.float32)
        nc.sync.dma_start(out=b_t[:], in_=bcol[:])
        io_t = const.tile([P, P], dt.float16)
        nc.sync.dma_start(out=io_t[:], in_=iot[:])
        io_b = io_t[:].unsqueeze(1).broadcast_to([P, SPLIT, P])

        def load_sb(sb):
            """Input tiles + gathers for super-block sb. Issued one SB ahead
            of the compute so the loads never queue behind the out write."""
            ix = ixpool.tile([P, NGROUP * P], dt.int16, tag="ix")
            nc.sync.dma_start(out=ix[:], in_=ixd[sb])
            mt = mpool.tile([P, SB * MCOLS], dt.float16, tag="mt")
            nc.scalar.dma_start(out=mt[:], in_=meta[sb])
            zs = zspool.tile([P, SB * P], dt.float16, tag="zs")
            nc.scalar.dma_start(out=zs[:], in_=zsb[sb])
            sst = strpool.tile([P, SB * NSTR * P], dt.float16, tag="sst")
            nc.scalar.dma_start(out=sst[:], in_=sdram[sb])
            gws = []
            for g in range(NGROUP):
                g_w = gpools[g].tile([P, GTILES * P], dt.float16, tag=f"G{g}")
                nc.gpsimd.dma_gather(
                    out_ap=g_w[:].rearrange("p (j n) -> p j n", n=P),
                    in_ap=zt[g * GROWS:(g + 1) * GROWS, :],
                    idxs_ap=ix[:, g * P:(g + 1) * P],
                    num_idxs=NIDX,
                    num_idxs_reg=NIDX,
                    elem_size=P,
                    queue_num=g,
                    single_packet=False,
                )
                gws.append(g_w)
            return mt, zs, sst, gws

        cur = load_sb(0)
        nxt = load_sb(1)
        for sb in range(NSB):
            mt, zs, sst, gws = cur

            ob = opool.tile([P, SB * P], dt.float16, tag="ob")
            for b in range(SB):
                c0 = b * MCOLS
                mslot = mt[:, c0:c0 + SPLIT].unsqueeze(2).broadcast_to(
                    [P, SPLIT, P])
                mw = mt[:, c0 + SPLIT:c0 + 2 * SPLIT].unsqueeze(2).broadcast_to(
                    [P, SPLIT, P])

                s_w = spool.tile([P, SPLIT * P], dt.float16, tag="S")
                s3 = s_w[:].rearrange("p (t n) -> p t n", n=P)
                nc.vector.tensor_tensor(out=s3, in0=io_b, in1=mslot, op=eq)
                nc.vector.tensor_tensor(out=s3, in0=s3, in1=mw, op=mul)

                def s_tile(gt):
                    if gt < SPLIT:
                        return s_w[:, gt * P:(gt + 1) * P]
                    j = gt - SPLIT
                    return sst[:, (b * NSTR + j) * P:(b * NSTR + j + 1) * P]

                psum = ppool.tile([P, P], dt.float32, tag="psA")
                for gt in range(TB):
                    nc.tensor.matmul(
                        out=psum[:],
                        lhsT=gws[gt // TPG][:, (b * TPG + gt % TPG) * P:
                                            (b * TPG + gt % TPG + 1) * P],
                        rhs=s_tile(gt),
                        start=(gt == 0),
                        stop=False,
                    )
                nc.tensor.matmul(
                    out=psum[:],
                    lhsT=zs[:, b * P:(b + 1) * P],
                    rhs=s_tile(TB),
                    start=False,
                    stop=True,
                )

                agg = apool.tile([P, P], dt.float16, tag="agg")
                nc.scalar.activation(out=agg[:], in_=psum[:],
                                     func=mybir.ActivationFunctionType.Copy)

                psum2 = p2pool.tile([P, P], dt.float32, tag="psB")
                nc.tensor.matmul(out=psum2[:], lhsT=w_t[:], rhs=agg[:],
                                 start=True, stop=True)
                nc.scalar.activation(
                    out=ob[:, b * P:(b + 1) * P],
                    in_=psum2[:],
                    func=mybir.ActivationFunctionType.Identity,
                    bias=b_t[:, 0:1],
                )
            nc.sync.dma_start(out=out[:, sb * SB * P:(sb + 1) * SB * P],
                              in_=ob[:])
            cur = nxt
            if sb + 2 < NSB:
                nxt = load_sb(sb + 2)

    nc.compile()
    return nc


def preprocess(src, dst, ew):
    """Static (graph-only) device metadata + host spill list.

    Returns (ixd, meta, sdram, spill) where
      ixd:   [NCORES, NSB, P, NGROUP*P] int16 wrapped gather indices
      meta:  [NCORES, NSB, P, SB*MCOLS] fp16 (slot[SPLIT] | w[SPLIT])
      sdram: [NCORES, NSB, P, SB*NSTR*P] fp16 host-built one-hot tiles
             (gt = SPLIT..TB-1) plus the self-loop diagonal tile
      spill: (src, dst, w) of edges beyond each cell's 512 capacity
    """
    deg = np.bincount(dst, weights=ew.astype(np.float64),
                      minlength=N_NODES) + 1.0
    dinv = (1.0 / np.sqrt(deg)).astype(np.float32)
    wtil = (dinv[src] * ew.astype(np.float32) * dinv[dst]).astype(np.float32)
    wself = np.zeros(N_PAD, np.float32)
    wself[:N_NODES] = dinv * dinv

    cell = (dst // P) * NGROUP + src // GROWS
    order = np.argsort(cell, kind="stable")
    cell_s = cell[order]
    src_s = src[order]
    dst_s = dst[order]
    w_s = wtil[order]

    ncells = NCORES * NB * NGROUP
    counts = np.bincount(cell_s, minlength=ncells)
    starts = np.zeros(ncells, np.int64)
    np.cumsum(counts[:-1], out=starts[1:])
    pos = np.arange(len(cell_s)) - starts[cell_s]
    keep = pos < CAP
    spill = (src_s[~keep], dst_s[~keep], w_s[~keep])

    ivec = np.zeros((ncells, CAP), np.int16)
    svec = np.zeros((ncells, CAP), np.int16)
    wvec = np.zeros((ncells, CAP), np.float16)
    flat = cell_s[keep] * CAP + pos[keep]
    ivec.reshape(-1)[flat] = (src_s[keep] % GROWS).astype(np.int16)
    svec.reshape(-1)[flat] = (dst_s[keep] % P).astype(np.int16)
    wvec.reshape(-1)[flat] = w_s[keep]

    # gather idx stream per (core, sb, g): 4 cells concatenated, wrapped in
    # 16 partitions (element i -> row i%16, col i//16), replicated 8x
    iv = ivec.reshape(NCORES, NSB, SB, NGROUP, CAP)
    iv = iv.transpose(0, 1, 3, 2, 4).reshape(NCORES, NSB, NGROUP, NIDX)
    ivw = iv.reshape(NCORES, NSB, NGROUP, NIDX // 16, 16).transpose(
        0, 1, 2, 4, 3)
    ivw = np.tile(ivw, (1, 1, 1, 8, 1))          # [c, sb, g, 128, 128]
    ixd = np.ascontiguousarray(ivw.transpose(0, 1, 3, 2, 4)).reshape(
        NCORES, NSB, P, NGROUP * P)

    # metadata: edge at gather position (b*512 + t*128 + p) of (sb, g) is
    # G tile column (b*TPG+t), partition p; S tile index gt = g*TPG+t.
    # Only the first SPLIT S tiles are built on-chip.
    sv = svec.reshape(NCORES, NSB, SB, NGROUP, TPG, P)
    smat = sv.transpose(0, 1, 2, 5, 3, 4).reshape(NCORES, NSB, SB, P, TB)
    wv = wvec.reshape(NCORES, NSB, SB, NGROUP, TPG, P)
    wmat = wv.transpose(0, 1, 2, 5, 3, 4).reshape(NCORES, NSB, SB, P, TB)
    meta = np.concatenate(
        [smat[..., :SPLIT].astype(np.float16),
         wmat[..., :SPLIT].astype(np.float16)], axis=4)
    meta = np.ascontiguousarray(meta.transpose(0, 1, 3, 2, 4)).reshape(
        NCORES, NSB, P, SB * MCOLS)

    # host-built streamed one-hot tiles gt = SPLIT..TB-1, plus diagonal
    nvals = np.arange(P, dtype=np.int16)[None, None, None, None, :]
    stiles = []
    for gt in range(SPLIT, TB):
        s_gt = smat[..., gt]                     # [c, nsb, sb, p]
        w_gt = wmat[..., gt]
        onehot = (s_gt[..., None] == nvals).astype(np.float16)
        onehot *= w_gt[..., None].astype(np.float16)
        stiles.append(onehot)                    # [c, nsb, sb, p, n]
    diag = (np.arange(P, dtype=np.int16)[:, None] == nvals[0, 0, 0]) \
        .astype(np.float16)                      # [p, n] identity
    wsm = wself.reshape(NCORES, NSB, SB, P).astype(np.float16)
    stiles.append(diag[None, None, None] * wsm[..., None])
    sarr = np.stack(stiles, axis=3)              # [c, nsb, sb, j, p, n]
    sdram = np.ascontiguousarray(sarr.transpose(0, 1, 4, 2, 3, 5)).reshape(
        NCORES, NSB, P, SB * NSTR * P)
    return ixd, meta, sdram, spill


def pack_zsb(z_f16):
    """Self-loop rows, 4 blocks per DMA: zsb[c, sb, p, b*128+f]."""
    z = z_f16.reshape(NCORES, NSB, SB, P, D)
    return np.ascontiguousarray(z.transpose(0, 1, 3, 2, 4)).reshape(
        NCORES, NSB, P, SB * D)


def run_layer(nc, z_f16, ixd, meta, sdram, W16, bcol, iota, *, trace=False):
    zsb = pack_zsb(z_f16)
    in_maps = []
    for c in range(NCORES):
        in_maps.append({
            "zt": z_f16,
            "ixd": ixd[c],
            "meta": meta[c],
            "sdram": sdram[c],
            "zsb": zsb[c],
            "wt": W16,
            "bcol": bcol,
            "iot": iota,
        })
    res = bass_utils.run_bass_kernel_spmd(
        nc, in_maps, core_ids=list(range(NCORES)), trace=trace,
    )
    # device emits pre-relu [f, nodes] per core; transpose to [nodes, f]
    pre = np.concatenate(
        [res.results[c]["out"].T for c in range(NCORES)], axis=0)
    return pre.astype(np.float32), res


def _enable_tracing():
    """Install the NTFF profile hook that this image's antenv lacks, and
    neuter the artifact upload (no bucket access here)."""
    import sys
    import types
    try:
        import antenv.axon_hooks  # noqa: F401
        have = True
    except ImportError:
        have = False
    if not have:
        mod = types.ModuleType("antenv.axon_hooks")
        mod._hook = None

        def set_axon_ntff_profile_hook(h):
            mod._hook = h

        def get_axon_ntff_profile_hook():
            return mod._hook

        mod.set_axon_ntff_profile_hook = set_axon_ntff_profile_hook
        mod.get_axon_ntff_profile_hook = get_axon_ntff_profile_hook
        sys.modules["antenv.axon_hooks"] = mod
        from trn_agent_boot.trn_boot import _ntff_profile_via_ctypes
        hook = _ntff_profile_via_ctypes("/opt/axon/libaxon_pjrt.so")
        mod.set_axon_ntff_profile_hook(hook)
    bass_utils.upload_artifacts = lambda tmpdir: f"local:{tmpdir}"


def _spill_agg(spill, z_f16):
    s_sp, d_sp, w_sp = spill
    acc = np.zeros((N_NODES, D), np.float32)
    if len(s_sp):
        np.add.at(acc, d_sp,
                  z_f16[s_sp].astype(np.float32) * w_sp[:, None])
    return acc


def kernel(x, edge_index, edge_weight, W1, b1, W2, b2):
    x = np.asarray(x, dtype=np.float32)
    edge_index = np.asarray(edge_index)
    edge_weight = np.asarray(edge_weight, dtype=np.float32)
    src = edge_index[0].astype(np.int64)
    dst = edge_index[1].astype(np.int64)

    ixd, meta, sdram, spill = preprocess(src, dst, edge_weight)

    if "nc" not in _nc_cache:
        _nc_cache["nc"] = build_nc()
    nc = _nc_cache["nc"]

    trace = bool(int(os.environ.get("GCN_TRACE", "0")))
    if trace:
        _enable_tracing()

    iota = np.ascontiguousarray(
        np.tile(np.arange(P, dtype=np.float16), (P, 1)))
    W1_16 = np.ascontiguousarray(W1.astype(np.float16))
    W2_16 = np.ascontiguousarray(W2.astype(np.float16))
    b1c = np.ascontiguousarray(b1.astype(np.float32).reshape(D, 1))
    b2c = np.ascontiguousarray(b2.astype(np.float32).reshape(D, 1))

    z1 = np.zeros((N_PAD, D), np.float16)
    z1[:N_NODES] = x.astype(np.float16)
    pre1, res1 = run_layer(nc, z1, ixd, meta, sdram, W1_16, b1c, iota,
                           trace=trace)
    pre1[:N_NODES] += _spill_agg(spill, z1) @ W1.astype(np.float32)
    h1 = np.maximum(pre1, 0.0)

    z2 = np.zeros((N_PAD, D), np.float16)
    z2[:N_NODES] = h1[:N_NODES].astype(np.float16)
    pre2, res2 = run_layer(nc, z2, ixd, meta, sdram, W2_16, b2c, iota,
                           trace=trace)
    pre2[:N_NODES] += _spill_agg(spill, z2) @ W2.astype(np.float32)
    h2 = np.maximum(pre2[:N_NODES], 0.0)

    if trace:
        t1 = res1.exec_time_ns or 0
        t2 = res2.exec_time_ns or 0
        print(f"[kernel] layer1 exec: {t1} ns, layer2 exec: {t2} ns, "
              f"total: {t1 + t2} ns")
        kernel.last_exec_ns = t1 + t2
        kernel.last_results = (res1, res2)

    return h2.astype(np.float32)
